# revision 23
# baseline (speedup 1.0000x reference)
"""Bass/Trainium2 kernel for LocalAttention (block-diagonal MHA, causal).

Model: x[B=4, SEQ=4096, D=1024] split into SPLIT=4 sequence blocks of L=1024,
each with its own MHA weights (H=16 heads, DK=64), causal within block.

Sharding: 16 (batch, split) blocks over 8 cores; core i handles split i//2 and
batches {2*(i%2), 2*(i%2)+1}. No collectives.

Per-core program (v3, precision-safe bf16 compute):
  - wq/wk bulk-loaded once; wv/wo streamed in 512-col chunks per block.
  - Q/K projections: W stationary, x^T moving, out qt/kt [e, l] bf16 per-et
    tiles (Q bias fused into the PSUM->SBUF copy; K bias dropped - it is a
    per-query constant, softmax-invariant).
  - Scores q^T k bf16 per lk-tile, exact causal trim; exp on ACT with fused
    scale 1/(32*32*sqrt(DK)) writing bf16 P^T; diagonal blocks masked by one
    strided bf16 multiply per (head, j).
  - PV bf16 with a 32.0 ones-column in V_aug giving the softmax denominator
    in psum row 64; denominator row -> reciprocal -> gpsimd partition
    broadcast; the normalize-multiply doubles as the PSUM->SBUF bf16 move
    and is software-pipelined one unit behind (never holds the DVE queue).
  - PV of head h runs behind the scores of head h+1 so the PE stream never
    waits on exp/mask.
  - Out-projection bf16; bias bo' = bv@Wo + bo added via broadcast row.
"""

import numpy as np
import ml_dtypes
from contextlib import ExitStack

import concourse.bass as bass
import concourse.bacc as bacc
import concourse.mybir as mybir
import concourse.tile as tile
from concourse.bass_utils import run_bass_kernel_spmd

H = 16
D = 1024
SPLIT = 4
DK = 64
B = 4
SEQ = 4096
L = SEQ // SPLIT          # 1024 tokens per block
NB = 2                    # blocks per core
NCORES = 8
NT = D // 128             # 8 e-tiles / d-tiles
NLT = L // 128            # 8 lk-tiles
F32 = mybir.dt.float32
BF16 = mybir.dt.bfloat16
WS = 32.0                 # q/k scale (folded out in the exp scale)
EXPS = 1.0 / (WS * WS * 8.0)


def build_program():
    nc = bacc.Bacc()

    xt_d = nc.declare_dram_parameter("xt", [NB, 128, NT, L], BF16, isOutput=False)
    wq_d = nc.declare_dram_parameter("wq", [NT, 128, NT, 128], BF16, isOutput=False)
    wk_d = nc.declare_dram_parameter("wk", [NT, 128, NT, 128], BF16, isOutput=False)
    wv_d = nc.declare_dram_parameter("wv", [128, NT, D], BF16, isOutput=False)
    wo_d = nc.declare_dram_parameter("wo", [128, NT, D], BF16, isOutput=False)
    bq_d = nc.declare_dram_parameter("bq", [128, NT], F32, isOutput=False)
    bop_d = nc.declare_dram_parameter("bop", [1, D], F32, isOutput=False)
    mask_d = nc.declare_dram_parameter("mask", [128, 128], BF16, isOutput=False)
    out_d = nc.declare_dram_parameter("out", [NB, NLT, 128, D], F32, isOutput=True)

    with ExitStack() as ctx:
        tc = ctx.enter_context(tile.TileContext(nc))
        consts = ctx.enter_context(tc.tile_pool(name="consts", bufs=1))
        wpool = ctx.enter_context(tc.tile_pool(name="wpool", bufs=1))
        wmov = ctx.enter_context(tc.tile_pool(name="wmov", bufs=6))
        xp = ctx.enter_context(tc.tile_pool(name="xp", bufs=2))
        qk = ctx.enter_context(tc.tile_pool(name="qk", bufs=1))
        va_p = ctx.enter_context(tc.tile_pool(name="va", bufs=2))
        ot_p = ctx.enter_context(tc.tile_pool(name="ot", bufs=2))
        pt_p = ctx.enter_context(tc.tile_pool(name="pt", bufs=2))
        rec_p = ctx.enter_context(tc.tile_pool(name="rec", bufs=2))
        stg_p = ctx.enter_context(tc.tile_pool(name="stg", bufs=2))
        osb_p = ctx.enter_context(tc.tile_pool(name="osb", bufs=2))
        scr_p = ctx.enter_context(tc.tile_pool(name="scr", bufs=4, space="DRAM"))
        ps = ctx.enter_context(tc.tile_pool(name="ps", bufs=2, space="PSUM"))

        # ---- constants & resident weights (once) ------------------------
        bq_sb = consts.tile([128, NT], F32, tag="bq")
        mask_sb = consts.tile([128, 128], BF16, tag="mask")
        bo_bc = consts.tile([128, D], F32, tag="bobc")
        nc.sync.dma_start(out=bq_sb, in_=bq_d[:, :])
        nc.sync.dma_start(out=mask_sb, in_=mask_d[:, :])
        bop_bcast = bass.AP(tensor=bop_d, offset=0, ap=[[0, 128], [1, D]])
        nc.sync.dma_start(out=bo_bc, in_=bop_bcast)


        xts = []
        for blk in range(NB):
            xt = xp.tile([128, NT, L], BF16, name=f"xt{blk}", tag="xt")
            nc.sync.dma_start(out=xt, in_=xt_d[blk])
            xts.append(xt)

        for blk in range(NB):
            xt = xts[blk]

            # ---- Q/K projections, out [e, l] bf16, per-et tiles ---------
            qts, kts = [], []
            for et in range(NT):
                for w_d, b_ap, lst, nm in ((wq_d, bq_sb, qts, "q"),
                                           (wk_d, None, kts, "k")):
                    o_sb = qk.tile([128, L], BF16, name=f"{nm}t{et}",
                                   tag=f"{nm}t{et}")
                    lst.append(o_sb)
                    wce = wmov.tile([128, NT, 128], BF16, name="wqkc",
                                    tag="wqkc", bufs=4)
                    nc.sync.dma_start(out=wce, in_=w_d[et])
                    for j in range(2):
                        psq = ps.tile([128, 512], F32, tag="psa", bufs=2)
                        for d in range(NT):
                            nc.tensor.matmul(
                                psq[:, :], wce[:, d, :],
                                xt[:, d, j * 512:(j + 1) * 512],
                                start=(d == 0), stop=(d == NT - 1))
                        dst = o_sb[:, j * 512:(j + 1) * 512]
                        if b_ap is not None:
                            nc.vector.tensor_scalar_add(
                                out=dst, in0=psq, scalar1=b_ap[:, et:et + 1])
                        else:
                            nc.vector.tensor_copy(out=dst, in_=psq)

            # ---- V projection into V_aug bf16 ---------------------------
            # va[:, lt, h, 0:64] = 32*v[lk, e in head h]; col 64 = 32.0
            va = va_p.tile([128, NLT, H, DK + 1], BF16, tag="va")
            nc.vector.memset(va[:, :, :, DK:DK + 1], WS)
            for g in range(2):
                chunks = []
                for d in range(NT):
                    wc = wmov.tile([128, 512], BF16, tag="wvc", bufs=10)
                    nc.sync.dma_start(out=wc, in_=wv_d[:, d, g * 512:(g + 1) * 512])
                    chunks.append(wc)
                for lt in range(NLT):
                    psv = ps.tile([128, 512], F32, tag="psa", bufs=2)
                    for d in range(NT):
                        nc.tensor.matmul(
                            psv[:, :], xt[:, d, lt * 128:(lt + 1) * 128],
                            chunks[d][:, :],
                            start=(d == 0), stop=(d == NT - 1))
                    nc.scalar.copy(
                        out=va[:, lt, g * 8:(g + 1) * 8, 0:DK],
                        in_=psv.rearrange("p (h k) -> p h k", h=8))

            # ---- attention ---------------------------------------------
            ot = ot_p.tile([128, NT, L], BF16, tag="ot")
            pending_mul = []

            def mul_stage():
                h, j, op, rec = pending_mul.pop(0)
                et, half = h // 2, h % 2
                p0 = DK * half
                dst_cols = slice(j * 512, (j + 1) * 512)
                if half == 0:
                    nc.vector.tensor_mul(
                        out=ot[0:DK, et, dst_cols],
                        in0=op[0:DK, :], in1=rec[0:DK, :])
                else:
                    stg = stg_p.tile([DK, 512], BF16, tag="stg")
                    nc.vector.tensor_mul(out=stg, in0=op[0:DK, :],
                                         in1=rec[0:DK, :])
                    nc.sync.dma_start(
                        out=ot[p0:p0 + DK, et, dst_cols], in_=stg)

            def pv_stage(h, j, pt):
                op = ps.tile([DK + 1, 512], F32, tag="pvs", bufs=2)
                tiles = [(i, max(0, 128 * (i - 4 * j))) for i in range(4 * j + 4)]
                for n, (i, lo) in enumerate(tiles):
                    nc.tensor.matmul(
                        op[0:DK + 1, lo:512],
                        va[:, i, h, 0:DK + 1],
                        pt[:, i, lo:512],
                        start=(n == 0), stop=(n == len(tiles) - 1))
                # sums row (psum p64) -> SBUF row -> recip -> broadcast;
                # normalize-mul runs one unit later (never holds DVE on
                # the Pool broadcast)
                rec = rec_p.tile([DK + 1, 512], F32, tag="rec")
                nc.vector.tensor_copy(out=rec[DK:DK + 1, :],
                                      in_=op[DK:DK + 1, :])
                nc.vector.reciprocal(out=rec[DK:DK + 1, :],
                                     in_=rec[DK:DK + 1, :])
                scr = scr_p.tile([1, 512], F32, tag="scr")
                nc.sync.dma_start(out=scr, in_=rec[DK:DK + 1, :])
                scr_bc = bass.AP(tensor=scr.tensor, offset=scr.offset,
                                 ap=[[0, DK], [1, 512]])
                nc.sync.dma_start(out=rec[0:DK, :], in_=scr_bc)
                pending_mul.append((h, j, op, rec))
                if len(pending_mul) > 1:
                    mul_stage()

            pending = []
            for h in range(H):
                et, half = h // 2, h % 2
                p0 = DK * half
                qt, kt = qts[et], kts[et]
                pts = (pt_p.tile([128, 4, 512], BF16, name="pt0", tag="pt0", bufs=3),
                       pt_p.tile([128, NLT, 512], BF16, name="pt1", tag="pt1",
                                 bufs=3))
                for j in range(2):
                    pt = pts[j]
                    # diagonal band first so the mask can fire early
                    order = list(range(4 * j, 4 * j + 4)) + list(range(4 * j))
                    for n_i, i in enumerate(order):
                        d = i - 4 * j
                        lo = 128 * d if d > 0 else 0
                        sp = ps.tile([128, 512], F32, tag="sps", bufs=4)
                        nc.tensor.matmul(
                            sp[:, lo:512],
                            kt[p0:p0 + DK, i * 128:(i + 1) * 128],
                            qt[p0:p0 + DK, j * 512 + lo:(j + 1) * 512],
                            start=True, stop=True)
                        nc.scalar.activation(
                            out=pt[:, i, lo:512], in_=sp[:, lo:512],
                            func=mybir.ActivationFunctionType.Exp,
                            scale=EXPS)
                        if n_i == 3:
                            # mask the 4 diagonal tiles (offset of diag
                            # block i: 512*i + 128*(i-4j), stride 640)
                            diag = bass.AP(
                                tensor=pt.tensor,
                                offset=pt.offset + (2048 * j),
                                ap=[pt.ap[0], [640, 4], [1, 128]])
                            mask_bc = bass.AP(
                                tensor=mask_sb.tensor,
                                offset=mask_sb.offset,
                                ap=[mask_sb.ap[0], [0, 4], [1, 128]])
                            nc.vector.tensor_mul(out=diag, in0=diag,
                                                 in1=mask_bc)
                    # PV of the previous unit runs behind this unit's
                    # scores so the PE never waits on exp/mask
                    if pending:
                        pv_stage(*pending.pop(0))
                    pending.append((h, j, pt))
            while pending:
                pv_stage(*pending.pop(0))
            while pending_mul:
                mul_stage()

            # ---- output projection (bf16) ------------------------------
            for g in range(2):
                chunks = []
                for et in range(NT):
                    wc = wmov.tile([128, 512], BF16, tag="woc", bufs=10)
                    nc.sync.dma_start(out=wc, in_=wo_d[:, et, g * 512:(g + 1) * 512])
                    chunks.append(wc)
                for lt in range(NLT):
                    po = ps.tile([128, 512], F32, tag="psa", bufs=2)
                    for et in range(NT):
                        nc.tensor.matmul(
                            po[:, :], ot[:, et, lt * 128:(lt + 1) * 128],
                            chunks[et][:, :],
                            start=(et == 0), stop=(et == NT - 1))
                    osb = osb_p.tile([128, 512], F32, tag="osb")
                    nc.vector.tensor_add(out=osb, in0=po,
                                        in1=bo_bc[:, g * 512:(g + 1) * 512])
                    nc.sync.dma_start(
                        out=out_d[blk, lt, :, g * 512:(g + 1) * 512],
                        in_=osb)
    nc.compile()
    return nc


def _prep_core_inputs(core, x, Wq, Wk, Wv, Wo, bq, bk, bv, bo, mask_bf):
    bf = ml_dtypes.bfloat16
    s = core // 2
    bs = (2 * (core % 2), 2 * (core % 2) + 1)
    xt = np.empty((NB, 128, NT, L), bf)
    for n, b in enumerate(bs):
        xb = np.ascontiguousarray(x[b, s * L:(s + 1) * L, :].T)      # [D, L]
        xt[n] = xb.reshape(NT, 128, L).transpose(1, 0, 2).astype(bf)
    # stationary W tiles: wq[p, d, et, m] = 32*Wq[d*128+p, et*128+m]
    wq = np.ascontiguousarray(
        (Wq[s] * WS).reshape(NT, 128, NT, 128).transpose(2, 1, 0, 3)).astype(bf)
    wk = np.ascontiguousarray(
        (Wk[s] * WS).reshape(NT, 128, NT, 128).transpose(2, 1, 0, 3)).astype(bf)
    wv = np.ascontiguousarray(
        (Wv[s] * WS).reshape(NT, 128, D).transpose(1, 0, 2)).astype(bf)
    wo = np.ascontiguousarray(
        Wo[s].reshape(NT, 128, D).transpose(1, 0, 2)).astype(bf)
    bq_t = np.ascontiguousarray((bq[s] * WS).reshape(NT, 128).T).astype(np.float32)
    bop = (bv[s] @ Wo[s] + bo[s]).reshape(1, D).astype(np.float32)
    return {"xt": xt, "wq": wq, "wk": wk, "wv": wv, "wo": wo,
            "bq": bq_t, "bop": bop, "mask": mask_bf}


_PROGRAM_CACHE = {}


def run(x, Wq, Wk, Wv, Wo, bq, bk, bv, bo, trace=False, **run_kwargs):
    x = np.asarray(x, np.float32)
    Wq, Wk, Wv, Wo = (np.asarray(a, np.float32) for a in (Wq, Wk, Wv, Wo))
    bq, bk, bv, bo = (np.asarray(a, np.float32) for a in (bq, bk, bv, bo))
    mask_bf = np.triu(np.ones((128, 128))).astype(ml_dtypes.bfloat16)

    if "nc" not in _PROGRAM_CACHE:
        _PROGRAM_CACHE["nc"] = build_program()
    nc = _PROGRAM_CACHE["nc"]

    in_maps = [_prep_core_inputs(c, x, Wq, Wk, Wv, Wo, bq, bk, bv, bo, mask_bf)
               for c in range(NCORES)]
    res = run_bass_kernel_spmd(nc, in_maps, core_ids=list(range(NCORES)),
                               trace=trace, **run_kwargs)
    out = np.empty((B, SEQ, D), np.float32)
    for c in range(NCORES):
        s = c // 2
        for n, b in enumerate((2 * (c % 2), 2 * (c % 2) + 1)):
            out[b, s * L:(s + 1) * L, :] = res.results[c]["out"][n].reshape(L, D)
    return out, res


def kernel(x, Wq, Wk, Wv, Wo, bq, bk, bv, bo):
    out, _ = run(x, Wq, Wk, Wv, Wo, bq, bk, bv, bo, trace=False)
    return out


# revision 31
# speedup vs baseline: 1.2683x; 1.2683x over previous
"""Bass/Trainium2 kernel for LocalAttention (block-diagonal MHA, causal).

Model: x[B=4, SEQ=4096, D=1024] split into SPLIT=4 sequence blocks of L=1024,
each with its own MHA weights (H=16 heads, DK=64), causal within block.

Sharding: 16 (batch, split) blocks over 8 cores; core i handles split i//2 and
batches {2*(i%2), 2*(i%2)+1}. No collectives.

Per-core program (v3, precision-safe bf16 compute):
  - wq/wk bulk-loaded once; wv/wo streamed in 512-col chunks per block.
  - Q/K projections: W stationary, x^T moving, out qt/kt [e, l] bf16 per-et
    tiles (Q bias fused into the PSUM->SBUF copy; K bias dropped - it is a
    per-query constant, softmax-invariant).
  - Scores q^T k bf16 per lk-tile, exact causal trim; exp on ACT with fused
    scale 1/(32*32*sqrt(DK)) writing bf16 P^T; diagonal blocks masked by one
    strided bf16 multiply per (head, j).
  - PV bf16 with a 32.0 ones-column in V_aug giving the softmax denominator
    in psum row 64; denominator row -> reciprocal -> gpsimd partition
    broadcast; the normalize-multiply doubles as the PSUM->SBUF bf16 move
    and is software-pipelined one unit behind (never holds the DVE queue).
  - PV of head h runs behind the scores of head h+1 so the PE stream never
    waits on exp/mask.
  - Out-projection bf16; bias bo' = bv@Wo + bo added via broadcast row.
"""

import numpy as np
import ml_dtypes
from contextlib import ExitStack

import concourse.bass as bass
import concourse.bacc as bacc
import concourse.mybir as mybir
import concourse.tile as tile
from concourse.bass_utils import run_bass_kernel_spmd

H = 16
D = 1024
SPLIT = 4
DK = 64
B = 4
SEQ = 4096
L = SEQ // SPLIT          # 1024 tokens per block
NB = 2                    # blocks per core
NCORES = 8
NT = D // 128             # 8 e-tiles / d-tiles
NLT = L // 128            # 8 lk-tiles
F32 = mybir.dt.float32
BF16 = mybir.dt.bfloat16
WS = 32.0                 # q/k scale (folded out in the exp scale)
EXPS = 1.0 / (WS * WS * 8.0)


def build_program():
    nc = bacc.Bacc()

    xt_d = nc.declare_dram_parameter("xt", [NB, 128, NT, L], BF16, isOutput=False)
    wq_d = nc.declare_dram_parameter("wq", [NT, 128, NT, 128], BF16, isOutput=False)
    wk_d = nc.declare_dram_parameter("wk", [NT, 128, NT, 128], BF16, isOutput=False)
    wv_d = nc.declare_dram_parameter("wv", [128, NT, D], BF16, isOutput=False)
    wo_d = nc.declare_dram_parameter("wo", [128, NT, D], BF16, isOutput=False)
    bq_d = nc.declare_dram_parameter("bq", [128, NT], F32, isOutput=False)
    bop_d = nc.declare_dram_parameter("bop", [1, D], F32, isOutput=False)
    mask_d = nc.declare_dram_parameter("mask", [128, 128], BF16, isOutput=False)
    out_d = nc.declare_dram_parameter("out", [NB, NLT, 128, D], F32, isOutput=True)

    with ExitStack() as ctx:
        tc = ctx.enter_context(tile.TileContext(nc))
        consts = ctx.enter_context(tc.tile_pool(name="consts", bufs=1))
        wpool = ctx.enter_context(tc.tile_pool(name="wpool", bufs=1))
        wmov = ctx.enter_context(tc.tile_pool(name="wmov", bufs=6))
        xp = ctx.enter_context(tc.tile_pool(name="xp", bufs=2))
        qk = ctx.enter_context(tc.tile_pool(name="qk", bufs=1))
        va_p = ctx.enter_context(tc.tile_pool(name="va", bufs=2))
        ot_p = ctx.enter_context(tc.tile_pool(name="ot", bufs=2))
        pt_p = ctx.enter_context(tc.tile_pool(name="pt", bufs=2))
        rec_p = ctx.enter_context(tc.tile_pool(name="rec", bufs=2))
        stg_p = ctx.enter_context(tc.tile_pool(name="stg", bufs=2))
        osb_p = ctx.enter_context(tc.tile_pool(name="osb", bufs=2))
        scr_p = ctx.enter_context(tc.tile_pool(name="scr", bufs=4, space="DRAM"))
        ps = ctx.enter_context(tc.tile_pool(name="ps", bufs=2, space="PSUM"))

        # ---- constants & resident weights (once) ------------------------
        bq_sb = consts.tile([128, NT], F32, tag="bq")
        mask_sb = consts.tile([128, 128], BF16, tag="mask")
        bo_bc = consts.tile([128, D], F32, tag="bobc")
        ones_sb = consts.tile([128, DK], BF16, tag="ones")
        nc.vector.memset(ones_sb, 1.0)
        nc.sync.dma_start(out=bq_sb, in_=bq_d[:, :])
        nc.sync.dma_start(out=mask_sb, in_=mask_d[:, :])
        bop_bcast = bass.AP(tensor=bop_d, offset=0, ap=[[0, 128], [1, D]])
        nc.sync.dma_start(out=bo_bc, in_=bop_bcast)


        xts = []
        for blk in range(NB):
            xt = xp.tile([128, NT, L], BF16, name=f"xt{blk}", tag="xt")
            nc.sync.dma_start(out=xt, in_=xt_d[blk])
            xts.append(xt)

        for blk in range(NB):
            xt = xts[blk]

            # ---- Q/K projections, out [e, l] bf16, per-et tiles ---------
            qts, kts = [], []
            for et in range(NT):
                for w_d, b_ap, lst, nm in ((wq_d, bq_sb, qts, "q"),
                                           (wk_d, None, kts, "k")):
                    o_sb = qk.tile([128, L], BF16, name=f"{nm}t{et}",
                                   tag=f"{nm}t{et}")
                    lst.append(o_sb)
                    wce = wmov.tile([128, NT, 128], BF16, name="wqkc",
                                    tag="wqkc", bufs=3)
                    nc.sync.dma_start(out=wce, in_=w_d[et])
                    for j in range(2):
                        psq = ps.tile([128, 512], F32, tag="psa", bufs=2)
                        for d in range(NT):
                            nc.tensor.matmul(
                                psq[:, :], wce[:, d, :],
                                xt[:, d, j * 512:(j + 1) * 512],
                                start=(d == 0), stop=(d == NT - 1))
                        dst = o_sb[:, j * 512:(j + 1) * 512]
                        if b_ap is not None:
                            nc.vector.tensor_scalar_add(
                                out=dst, in0=psq, scalar1=b_ap[:, et:et + 1])
                        else:
                            nc.vector.tensor_copy(out=dst, in_=psq)

            # ---- V projection into V_aug bf16 ---------------------------
            # va[:, lt, h, 0:64] = 32*v[lk, e in head h]; col 64 = 32.0
            va = va_p.tile([128, NLT, H, DK + 1], BF16, tag="va")
            nc.vector.memset(va[:, :, :, DK:DK + 1], WS)
            for g in range(2):
                chunks = []
                for d in range(NT):
                    wc = wmov.tile([128, 512], BF16, tag="wvc", bufs=10)
                    nc.sync.dma_start(out=wc, in_=wv_d[:, d, g * 512:(g + 1) * 512])
                    chunks.append(wc)
                for lt in range(NLT):
                    psv = ps.tile([128, 512], F32, tag="psa", bufs=2)
                    for d in range(NT):
                        nc.tensor.matmul(
                            psv[:, :], xt[:, d, lt * 128:(lt + 1) * 128],
                            chunks[d][:, :],
                            start=(d == 0), stop=(d == NT - 1))
                    nc.scalar.copy(
                        out=va[:, lt, g * 8:(g + 1) * 8, 0:DK],
                        in_=psv.rearrange("p (h k) -> p h k", h=8))

            # ---- attention ---------------------------------------------
            ot = ot_p.tile([128, NT, L], BF16, tag="ot")
            pending_nrm = []

            def nrm_stage():
                # PE broadcast of the reciprocal'd denominator row (K=1
                # f32r matmul: ones x row), then normalize out of psum.
                # Runs one unit behind its PV so the PE never waits on
                # the DVE reciprocal.
                h, j, op, rrow, rec = pending_nrm.pop(0)
                et, half = h // 2, h % 2
                p0 = DK * half
                rbc = ps.tile([DK, 512], F32, tag="rbc", bufs=1)
                nc.tensor.matmul(
                    rbc[:, :],
                    ones_sb[DK:DK + 1, 0:DK],
                    rrow[DK:DK + 1, :],
                    start=True, stop=True)
                nc.vector.tensor_copy(out=rec[0:DK, :], in_=rbc[:, :])
                dst_cols = slice(j * 512, (j + 1) * 512)
                if half == 0:
                    nc.vector.tensor_mul(
                        out=ot[0:DK, et, dst_cols],
                        in0=op[0:DK, :], in1=rec[0:DK, :])
                else:
                    stg = stg_p.tile([DK, 512], BF16, tag="stg")
                    nc.vector.tensor_mul(out=stg, in0=op[0:DK, :],
                                         in1=rec[0:DK, :])
                    nc.sync.dma_start(
                        out=ot[p0:p0 + DK, et, dst_cols], in_=stg)

            def pv_stage(h, j, pt):
                if pending_nrm:
                    nrm_stage()
                op = ps.tile([DK + 1, 512], F32, tag="pvs", bufs=2)
                tiles = [(i, max(0, 128 * (i - 4 * j))) for i in range(4 * j + 4)]
                for n, (i, lo) in enumerate(tiles):
                    nc.tensor.matmul(
                        op[0:DK + 1, lo:512],
                        va[:, i, h, 0:DK + 1],
                        pt[:, i, lo:512],
                        start=(n == 0), stop=(n == len(tiles) - 1))
                rrow = rec_p.tile([DK + 1, 512], BF16, tag="rrow")
                rec = rec_p.tile([DK, 512], F32, tag="rec")
                with nc.allow_low_precision("bf16 1/denominator (<=0.4%)"):
                    nc.vector.reciprocal(out=rrow[DK:DK + 1, :],
                                         in_=op[DK:DK + 1, :])
                pending_nrm.append((h, j, op, rrow, rec))

            pending = []
            for h in range(H):
                et, half = h // 2, h % 2
                p0 = DK * half
                qt, kt = qts[et], kts[et]
                pts = (pt_p.tile([128, 4, 512], BF16, name="pt0", tag="pt0", bufs=3),
                       pt_p.tile([128, NLT, 512], BF16, name="pt1", tag="pt1",
                                 bufs=3))
                for j in range(2):
                    pt = pts[j]
                    # diagonal band first so the mask can fire early
                    order = list(range(4 * j, 4 * j + 4)) + list(range(4 * j))
                    for n_i, i in enumerate(order):
                        d = i - 4 * j
                        lo = 128 * d if d > 0 else 0
                        sp = ps.tile([128, 512], F32, tag="sps", bufs=3)
                        nc.tensor.matmul(
                            sp[:, lo:512],
                            kt[p0:p0 + DK, i * 128:(i + 1) * 128],
                            qt[p0:p0 + DK, j * 512 + lo:(j + 1) * 512],
                            start=True, stop=True)
                        nc.scalar.activation(
                            out=pt[:, i, lo:512], in_=sp[:, lo:512],
                            func=mybir.ActivationFunctionType.Exp,
                            scale=EXPS)
                        if n_i == 3:
                            # mask the 4 diagonal tiles (offset of diag
                            # block i: 512*i + 128*(i-4j), stride 640)
                            diag = bass.AP(
                                tensor=pt.tensor,
                                offset=pt.offset + (2048 * j),
                                ap=[pt.ap[0], [640, 4], [1, 128]])
                            mask_bc = bass.AP(
                                tensor=mask_sb.tensor,
                                offset=mask_sb.offset,
                                ap=[mask_sb.ap[0], [0, 4], [1, 128]])
                            nc.vector.tensor_mul(out=diag, in0=diag,
                                                 in1=mask_bc)
                    # PV of the previous unit runs behind this unit's
                    # scores so the PE never waits on exp/mask
                    if pending:
                        pv_stage(*pending.pop(0))
                    pending.append((h, j, pt))
            while pending:
                pv_stage(*pending.pop(0))
            while pending_nrm:
                nrm_stage()

            # ---- output projection (bf16) ------------------------------
            for g in range(2):
                chunks = []
                for et in range(NT):
                    wc = wmov.tile([128, 512], BF16, tag="woc", bufs=10)
                    nc.sync.dma_start(out=wc, in_=wo_d[:, et, g * 512:(g + 1) * 512])
                    chunks.append(wc)
                for lt in range(NLT):
                    po = ps.tile([128, 512], F32, tag="psa", bufs=2)
                    for et in range(NT):
                        nc.tensor.matmul(
                            po[:, :], ot[:, et, lt * 128:(lt + 1) * 128],
                            chunks[et][:, :],
                            start=(et == 0), stop=(et == NT - 1))
                    osb = osb_p.tile([128, 512], F32, tag="osb")
                    nc.vector.tensor_add(out=osb, in0=po,
                                        in1=bo_bc[:, g * 512:(g + 1) * 512])
                    nc.sync.dma_start(
                        out=out_d[blk, lt, :, g * 512:(g + 1) * 512],
                        in_=osb)
    nc.compile()
    return nc


def _prep_core_inputs(core, x, Wq, Wk, Wv, Wo, bq, bk, bv, bo, mask_bf):
    bf = ml_dtypes.bfloat16
    s = core // 2
    bs = (2 * (core % 2), 2 * (core % 2) + 1)
    xt = np.empty((NB, 128, NT, L), bf)
    for n, b in enumerate(bs):
        xb = np.ascontiguousarray(x[b, s * L:(s + 1) * L, :].T)      # [D, L]
        xt[n] = xb.reshape(NT, 128, L).transpose(1, 0, 2).astype(bf)
    # stationary W tiles: wq[p, d, et, m] = 32*Wq[d*128+p, et*128+m]
    wq = np.ascontiguousarray(
        (Wq[s] * WS).reshape(NT, 128, NT, 128).transpose(2, 1, 0, 3)).astype(bf)
    wk = np.ascontiguousarray(
        (Wk[s] * WS).reshape(NT, 128, NT, 128).transpose(2, 1, 0, 3)).astype(bf)
    wv = np.ascontiguousarray(
        (Wv[s] * WS).reshape(NT, 128, D).transpose(1, 0, 2)).astype(bf)
    wo = np.ascontiguousarray(
        Wo[s].reshape(NT, 128, D).transpose(1, 0, 2)).astype(bf)
    bq_t = np.ascontiguousarray((bq[s] * WS).reshape(NT, 128).T).astype(np.float32)
    bop = (bv[s] @ Wo[s] + bo[s]).reshape(1, D).astype(np.float32)
    return {"xt": xt, "wq": wq, "wk": wk, "wv": wv, "wo": wo,
            "bq": bq_t, "bop": bop, "mask": mask_bf}


_PROGRAM_CACHE = {}


def run(x, Wq, Wk, Wv, Wo, bq, bk, bv, bo, trace=False, **run_kwargs):
    x = np.asarray(x, np.float32)
    Wq, Wk, Wv, Wo = (np.asarray(a, np.float32) for a in (Wq, Wk, Wv, Wo))
    bq, bk, bv, bo = (np.asarray(a, np.float32) for a in (bq, bk, bv, bo))
    mask_bf = np.triu(np.ones((128, 128))).astype(ml_dtypes.bfloat16)

    if "nc" not in _PROGRAM_CACHE:
        _PROGRAM_CACHE["nc"] = build_program()
    nc = _PROGRAM_CACHE["nc"]

    in_maps = [_prep_core_inputs(c, x, Wq, Wk, Wv, Wo, bq, bk, bv, bo, mask_bf)
               for c in range(NCORES)]
    res = run_bass_kernel_spmd(nc, in_maps, core_ids=list(range(NCORES)),
                               trace=trace, **run_kwargs)
    out = np.empty((B, SEQ, D), np.float32)
    for c in range(NCORES):
        s = c // 2
        for n, b in enumerate((2 * (c % 2), 2 * (c % 2) + 1)):
            out[b, s * L:(s + 1) * L, :] = res.results[c]["out"][n].reshape(L, D)
    return out, res


def kernel(x, Wq, Wk, Wv, Wo, bq, bk, bv, bo):
    out, _ = run(x, Wq, Wk, Wv, Wo, bq, bk, bv, bo, trace=False)
    return out


# revision 34
# speedup vs baseline: 1.2995x; 1.0246x over previous
"""Bass/Trainium2 kernel for LocalAttention (block-diagonal MHA, causal).

Model: x[B=4, SEQ=4096, D=1024] split into SPLIT=4 sequence blocks of L=1024,
each with its own MHA weights (H=16 heads, DK=64), causal within block.

Sharding: 16 (batch, split) blocks over 8 cores; core i handles split i//2 and
batches {2*(i%2), 2*(i%2)+1}. No collectives.

Per-core program (v3, precision-safe bf16 compute):
  - wq/wk bulk-loaded once; wv/wo streamed in 512-col chunks per block.
  - Q/K projections: W stationary, x^T moving, out qt/kt [e, l] bf16 per-et
    tiles (Q bias fused into the PSUM->SBUF copy; K bias dropped - it is a
    per-query constant, softmax-invariant).
  - Scores q^T k bf16 per lk-tile, exact causal trim; exp on ACT with fused
    scale 1/(32*32*sqrt(DK)) writing bf16 P^T; diagonal blocks masked by one
    strided bf16 multiply per (head, j).
  - PV bf16 with a 32.0 ones-column in V_aug giving the softmax denominator
    in psum row 64; denominator row -> reciprocal -> gpsimd partition
    broadcast; the normalize-multiply doubles as the PSUM->SBUF bf16 move
    and is software-pipelined one unit behind (never holds the DVE queue).
  - PV of head h runs behind the scores of head h+1 so the PE stream never
    waits on exp/mask.
  - Out-projection bf16; bias bo' = bv@Wo + bo added via broadcast row.
"""

import numpy as np
import ml_dtypes
from contextlib import ExitStack

import concourse.bass as bass
import concourse.bacc as bacc
import concourse.mybir as mybir
import concourse.tile as tile
from concourse.bass_utils import run_bass_kernel_spmd

H = 16
D = 1024
SPLIT = 4
DK = 64
B = 4
SEQ = 4096
L = SEQ // SPLIT          # 1024 tokens per block
NB = 2                    # blocks per core
NCORES = 8
NT = D // 128             # 8 e-tiles / d-tiles
NLT = L // 128            # 8 lk-tiles
F32 = mybir.dt.float32
BF16 = mybir.dt.bfloat16
WS = 32.0                 # q/k scale (folded out in the exp scale)
EXPS = 1.0 / (WS * WS * 8.0)


def build_program():
    nc = bacc.Bacc()

    xt_d = nc.declare_dram_parameter("xt", [NB, 128, NT, L], BF16, isOutput=False)
    wq_d = nc.declare_dram_parameter("wq", [NT, 128, NT, 128], BF16, isOutput=False)
    wk_d = nc.declare_dram_parameter("wk", [NT, 128, NT, 128], BF16, isOutput=False)
    wv_d = nc.declare_dram_parameter("wv", [128, NT, D], BF16, isOutput=False)
    wo_d = nc.declare_dram_parameter("wo", [128, NT, D], BF16, isOutput=False)
    bq_d = nc.declare_dram_parameter("bq", [128, NT], F32, isOutput=False)
    bop_d = nc.declare_dram_parameter("bop", [1, D], F32, isOutput=False)
    mask_d = nc.declare_dram_parameter("mask", [128, 128], BF16, isOutput=False)
    out_d = nc.declare_dram_parameter("out", [NB, NLT, 128, D], F32, isOutput=True)

    with ExitStack() as ctx:
        tc = ctx.enter_context(tile.TileContext(nc))
        consts = ctx.enter_context(tc.tile_pool(name="consts", bufs=1))
        wpool = ctx.enter_context(tc.tile_pool(name="wpool", bufs=1))
        wmov = ctx.enter_context(tc.tile_pool(name="wmov", bufs=6))
        xp = ctx.enter_context(tc.tile_pool(name="xp", bufs=2))
        qk = ctx.enter_context(tc.tile_pool(name="qk", bufs=1))
        va_p = ctx.enter_context(tc.tile_pool(name="va", bufs=2))
        ot_p = ctx.enter_context(tc.tile_pool(name="ot", bufs=2))
        pt_p = ctx.enter_context(tc.tile_pool(name="pt", bufs=2))
        rec_p = ctx.enter_context(tc.tile_pool(name="rec", bufs=2))
        stg_p = ctx.enter_context(tc.tile_pool(name="stg", bufs=2))
        osb_p = ctx.enter_context(tc.tile_pool(name="osb", bufs=2))
        scr_p = ctx.enter_context(tc.tile_pool(name="scr", bufs=4, space="DRAM"))
        ps = ctx.enter_context(tc.tile_pool(name="ps", bufs=2, space="PSUM"))

        # ---- constants & resident weights (once) ------------------------
        bq_sb = consts.tile([128, NT], F32, tag="bq")
        mask_sb = consts.tile([128, 128], BF16, tag="mask")
        bo_bc = consts.tile([128, D], F32, tag="bobc")
        ones_sb = consts.tile([128, DK], BF16, tag="ones")
        nc.vector.memset(ones_sb, 1.0)
        nc.sync.dma_start(out=bq_sb, in_=bq_d[:, :])
        nc.sync.dma_start(out=mask_sb, in_=mask_d[:, :])
        bop_bcast = bass.AP(tensor=bop_d, offset=0, ap=[[0, 128], [1, D]])
        nc.sync.dma_start(out=bo_bc, in_=bop_bcast)


        wce_pre = {}
        for w_d, nm in ((wq_d, "q"), (wk_d, "k")):
            wce = wmov.tile([128, NT, 128], BF16, name="wqkc", tag="wqkc",
                            bufs=3)
            nc.sync.dma_start(out=wce, in_=w_d[0])
            wce_pre[nm] = wce

        xts = []
        for blk in range(NB):
            xt = xp.tile([128, NT, L], BF16, name=f"xt{blk}", tag="xt")
            nc.sync.dma_start(out=xt[:, 0:4, :], in_=xt_d[blk, :, 0:4, :])
            nc.sync.dma_start(out=xt[:, 4:8, :], in_=xt_d[blk, :, 4:8, :])
            xts.append(xt)

        for blk in range(NB):
            xt = xts[blk]

            # ---- Q/K projections, out [e, l] bf16, per-et tiles ---------
            qts, kts = [], []
            for et in range(NT):
                for w_d, b_ap, lst, nm in ((wq_d, bq_sb, qts, "q"),
                                           (wk_d, None, kts, "k")):
                    o_sb = qk.tile([128, L], BF16, name=f"{nm}t{et}",
                                   tag=f"{nm}t{et}")
                    lst.append(o_sb)
                    if blk == 0 and et == 0:
                        wce = wce_pre[nm]
                    else:
                        wce = wmov.tile([128, NT, 128], BF16, name="wqkc",
                                        tag="wqkc", bufs=3)
                        nc.sync.dma_start(out=wce, in_=w_d[et])
                    for j in range(2):
                        psq = ps.tile([128, 512], F32, tag="psa", bufs=2)
                        for d in range(NT):
                            nc.tensor.matmul(
                                psq[:, :], wce[:, d, :],
                                xt[:, d, j * 512:(j + 1) * 512],
                                start=(d == 0), stop=(d == NT - 1))
                        dst = o_sb[:, j * 512:(j + 1) * 512]
                        if b_ap is not None:
                            nc.vector.tensor_scalar_add(
                                out=dst, in0=psq, scalar1=b_ap[:, et:et + 1])
                        else:
                            nc.vector.tensor_copy(out=dst, in_=psq)

            # ---- V projection into V_aug bf16 ---------------------------
            # va[:, lt, h, 0:64] = 32*v[lk, e in head h]; col 64 = 32.0
            va = va_p.tile([128, NLT, H, DK + 1], BF16, tag="va")
            nc.vector.memset(va[:, :, :, DK:DK + 1], WS)
            for g in range(2):
                chunks = []
                for d in range(NT):
                    wc = wmov.tile([128, 512], BF16, tag="wvc", bufs=10)
                    nc.sync.dma_start(out=wc, in_=wv_d[:, d, g * 512:(g + 1) * 512])
                    chunks.append(wc)
                for lt in range(NLT):
                    psv = ps.tile([128, 512], F32, tag="psa", bufs=2)
                    for d in range(NT):
                        nc.tensor.matmul(
                            psv[:, :], xt[:, d, lt * 128:(lt + 1) * 128],
                            chunks[d][:, :],
                            start=(d == 0), stop=(d == NT - 1))
                    nc.scalar.copy(
                        out=va[:, lt, g * 8:(g + 1) * 8, 0:DK],
                        in_=psv.rearrange("p (h k) -> p h k", h=8))

            # ---- attention ---------------------------------------------
            ot = ot_p.tile([128, NT, L], BF16, tag="ot")
            pending_nrm = []

            def nrm_stage():
                # PE broadcast of the reciprocal'd denominator row (K=1
                # f32r matmul: ones x row), then normalize out of psum.
                # Runs one unit behind its PV so the PE never waits on
                # the DVE reciprocal.
                h, j, op, rrow, rec = pending_nrm.pop(0)
                et, half = h // 2, h % 2
                p0 = DK * half
                rbc = ps.tile([DK, 512], F32, tag="rbc", bufs=1)
                nc.tensor.matmul(
                    rbc[:, :],
                    ones_sb[DK:DK + 1, 0:DK],
                    rrow[DK:DK + 1, :],
                    start=True, stop=True)
                nc.vector.tensor_copy(out=rec[0:DK, :], in_=rbc[:, :])
                dst_cols = slice(j * 512, (j + 1) * 512)
                if half == 0:
                    nc.vector.tensor_mul(
                        out=ot[0:DK, et, dst_cols],
                        in0=op[0:DK, :], in1=rec[0:DK, :])
                else:
                    stg = stg_p.tile([DK, 512], BF16, tag="stg")
                    nc.vector.tensor_mul(out=stg, in0=op[0:DK, :],
                                         in1=rec[0:DK, :])
                    nc.sync.dma_start(
                        out=ot[p0:p0 + DK, et, dst_cols], in_=stg)

            def pv_stage(h, j, pt):
                if pending_nrm:
                    nrm_stage()
                op = ps.tile([DK + 1, 512], F32, tag="pvs", bufs=2)
                tiles = [(i, max(0, 128 * (i - 4 * j))) for i in range(4 * j + 4)]
                for n, (i, lo) in enumerate(tiles):
                    nc.tensor.matmul(
                        op[0:DK + 1, lo:512],
                        va[:, i, h, 0:DK + 1],
                        pt[:, i, lo:512],
                        start=(n == 0), stop=(n == len(tiles) - 1))
                rrow = rec_p.tile([DK + 1, 512], BF16, tag="rrow")
                rec = rec_p.tile([DK, 512], F32, tag="rec")
                with nc.allow_low_precision("bf16 1/denominator (<=0.4%)"):
                    nc.vector.reciprocal(out=rrow[DK:DK + 1, :],
                                         in_=op[DK:DK + 1, :])
                pending_nrm.append((h, j, op, rrow, rec))

            pending = []
            for h in range(H):
                et, half = h // 2, h % 2
                p0 = DK * half
                qt, kt = qts[et], kts[et]
                pts = (pt_p.tile([128, 4, 512], BF16, name="pt0", tag="pt0", bufs=3),
                       pt_p.tile([128, NLT, 512], BF16, name="pt1", tag="pt1",
                                 bufs=3))
                for j in range(2):
                    pt = pts[j]
                    # diagonal band first so the mask can fire early
                    order = list(range(4 * j, 4 * j + 4)) + list(range(4 * j))
                    for n_i, i in enumerate(order):
                        d = i - 4 * j
                        lo = 128 * d if d > 0 else 0
                        sp = ps.tile([128, 512], F32, tag="sps", bufs=3)
                        nc.tensor.matmul(
                            sp[:, lo:512],
                            kt[p0:p0 + DK, i * 128:(i + 1) * 128],
                            qt[p0:p0 + DK, j * 512 + lo:(j + 1) * 512],
                            start=True, stop=True)
                        nc.scalar.activation(
                            out=pt[:, i, lo:512], in_=sp[:, lo:512],
                            func=mybir.ActivationFunctionType.Exp,
                            scale=EXPS)
                        if n_i == 3:
                            # mask the 4 diagonal tiles (offset of diag
                            # block i: 512*i + 128*(i-4j), stride 640)
                            diag = bass.AP(
                                tensor=pt.tensor,
                                offset=pt.offset + (2048 * j),
                                ap=[pt.ap[0], [640, 4], [1, 128]])
                            mask_bc = bass.AP(
                                tensor=mask_sb.tensor,
                                offset=mask_sb.offset,
                                ap=[mask_sb.ap[0], [0, 4], [1, 128]])
                            nc.vector.tensor_mul(out=diag, in0=diag,
                                                 in1=mask_bc)
                    # PV of the previous unit runs behind this unit's
                    # scores so the PE never waits on exp/mask
                    if pending:
                        pv_stage(*pending.pop(0))
                    pending.append((h, j, pt))
            while pending:
                pv_stage(*pending.pop(0))
            while pending_nrm:
                nrm_stage()

            # ---- output projection (bf16) ------------------------------
            for g in range(2):
                chunks = []
                for et in range(NT):
                    wc = wmov.tile([128, 512], BF16, tag="woc", bufs=10)
                    nc.sync.dma_start(out=wc, in_=wo_d[:, et, g * 512:(g + 1) * 512])
                    chunks.append(wc)
                for lt in range(NLT):
                    po = ps.tile([128, 512], F32, tag="psa", bufs=2)
                    for et in range(NT):
                        nc.tensor.matmul(
                            po[:, :], ot[:, et, lt * 128:(lt + 1) * 128],
                            chunks[et][:, :],
                            start=(et == 0), stop=(et == NT - 1))
                    osb = osb_p.tile([128, 512], F32, tag="osb")
                    nc.vector.tensor_add(out=osb, in0=po,
                                        in1=bo_bc[:, g * 512:(g + 1) * 512])
                    nc.sync.dma_start(
                        out=out_d[blk, lt, :, g * 512:(g + 1) * 512],
                        in_=osb)
    nc.compile()
    return nc


def _prep_core_inputs(core, x, Wq, Wk, Wv, Wo, bq, bk, bv, bo, mask_bf):
    bf = ml_dtypes.bfloat16
    s = core // 2
    bs = (2 * (core % 2), 2 * (core % 2) + 1)
    xt = np.empty((NB, 128, NT, L), bf)
    for n, b in enumerate(bs):
        xb = np.ascontiguousarray(x[b, s * L:(s + 1) * L, :].T)      # [D, L]
        xt[n] = xb.reshape(NT, 128, L).transpose(1, 0, 2).astype(bf)
    # stationary W tiles: wq[p, d, et, m] = 32*Wq[d*128+p, et*128+m]
    wq = np.ascontiguousarray(
        (Wq[s] * WS).reshape(NT, 128, NT, 128).transpose(2, 1, 0, 3)).astype(bf)
    wk = np.ascontiguousarray(
        (Wk[s] * WS).reshape(NT, 128, NT, 128).transpose(2, 1, 0, 3)).astype(bf)
    wv = np.ascontiguousarray(
        (Wv[s] * WS).reshape(NT, 128, D).transpose(1, 0, 2)).astype(bf)
    wo = np.ascontiguousarray(
        Wo[s].reshape(NT, 128, D).transpose(1, 0, 2)).astype(bf)
    bq_t = np.ascontiguousarray((bq[s] * WS).reshape(NT, 128).T).astype(np.float32)
    bop = (bv[s] @ Wo[s] + bo[s]).reshape(1, D).astype(np.float32)
    return {"xt": xt, "wq": wq, "wk": wk, "wv": wv, "wo": wo,
            "bq": bq_t, "bop": bop, "mask": mask_bf}


_PROGRAM_CACHE = {}


def run(x, Wq, Wk, Wv, Wo, bq, bk, bv, bo, trace=False, **run_kwargs):
    x = np.asarray(x, np.float32)
    Wq, Wk, Wv, Wo = (np.asarray(a, np.float32) for a in (Wq, Wk, Wv, Wo))
    bq, bk, bv, bo = (np.asarray(a, np.float32) for a in (bq, bk, bv, bo))
    mask_bf = np.triu(np.ones((128, 128))).astype(ml_dtypes.bfloat16)

    if "nc" not in _PROGRAM_CACHE:
        _PROGRAM_CACHE["nc"] = build_program()
    nc = _PROGRAM_CACHE["nc"]

    in_maps = [_prep_core_inputs(c, x, Wq, Wk, Wv, Wo, bq, bk, bv, bo, mask_bf)
               for c in range(NCORES)]
    res = run_bass_kernel_spmd(nc, in_maps, core_ids=list(range(NCORES)),
                               trace=trace, **run_kwargs)
    out = np.empty((B, SEQ, D), np.float32)
    for c in range(NCORES):
        s = c // 2
        for n, b in enumerate((2 * (c % 2), 2 * (c % 2) + 1)):
            out[b, s * L:(s + 1) * L, :] = res.results[c]["out"][n].reshape(L, D)
    return out, res


def kernel(x, Wq, Wk, Wv, Wo, bq, bk, bv, bo):
    out, _ = run(x, Wq, Wk, Wv, Wo, bq, bk, bv, bo, trace=False)
    return out


# revision 35
# speedup vs baseline: 1.3535x; 1.0415x over previous
"""Bass/Trainium2 kernel for LocalAttention (block-diagonal MHA, causal).

Model: x[B=4, SEQ=4096, D=1024] split into SPLIT=4 sequence blocks of L=1024,
each with its own MHA weights (H=16 heads, DK=64), causal within block.

Sharding: 16 (batch, split) blocks over 8 cores; core i handles split i//2 and
batches {2*(i%2), 2*(i%2)+1}. No collectives.

Per-core program (v3, precision-safe bf16 compute):
  - wq/wk bulk-loaded once; wv/wo streamed in 512-col chunks per block.
  - Q/K projections: W stationary, x^T moving, out qt/kt [e, l] bf16 per-et
    tiles (Q bias fused into the PSUM->SBUF copy; K bias dropped - it is a
    per-query constant, softmax-invariant).
  - Scores q^T k bf16 per lk-tile, exact causal trim; exp on ACT with fused
    scale 1/(32*32*sqrt(DK)) writing bf16 P^T; diagonal blocks masked by one
    strided bf16 multiply per (head, j).
  - PV bf16 with a 32.0 ones-column in V_aug giving the softmax denominator
    in psum row 64; denominator row -> reciprocal -> gpsimd partition
    broadcast; the normalize-multiply doubles as the PSUM->SBUF bf16 move
    and is software-pipelined one unit behind (never holds the DVE queue).
  - PV of head h runs behind the scores of head h+1 so the PE stream never
    waits on exp/mask.
  - Out-projection bf16; bias bo' = bv@Wo + bo added via broadcast row.
"""

import numpy as np
import ml_dtypes
from contextlib import ExitStack

import concourse.bass as bass
import concourse.bacc as bacc
import concourse.mybir as mybir
import concourse.tile as tile
from concourse.bass_utils import run_bass_kernel_spmd

H = 16
D = 1024
SPLIT = 4
DK = 64
B = 4
SEQ = 4096
L = SEQ // SPLIT          # 1024 tokens per block
NB = 2                    # blocks per core
NCORES = 8
NT = D // 128             # 8 e-tiles / d-tiles
NLT = L // 128            # 8 lk-tiles
F32 = mybir.dt.float32
BF16 = mybir.dt.bfloat16
WS = 32.0                 # q/k scale (folded out in the exp scale)
EXPS = 1.0 / (WS * WS * 8.0)


def build_program():
    nc = bacc.Bacc()

    xt_d = nc.declare_dram_parameter("xt", [NB, 128, NT, L], BF16, isOutput=False)
    wq_d = nc.declare_dram_parameter("wq", [NT, 128, NT, 128], BF16, isOutput=False)
    wk_d = nc.declare_dram_parameter("wk", [NT, 128, NT, 128], BF16, isOutput=False)
    wv_d = nc.declare_dram_parameter("wv", [128, NT, D], BF16, isOutput=False)
    wo_d = nc.declare_dram_parameter("wo", [128, NT, D], BF16, isOutput=False)
    bq_d = nc.declare_dram_parameter("bq", [128, NT], F32, isOutput=False)
    bop_d = nc.declare_dram_parameter("bop", [1, D], F32, isOutput=False)
    mask_d = nc.declare_dram_parameter("mask", [128, 128], BF16, isOutput=False)
    out_d = nc.declare_dram_parameter("out", [NB, NLT, 128, D], F32, isOutput=True)

    with ExitStack() as ctx:
        tc = ctx.enter_context(tile.TileContext(nc))
        consts = ctx.enter_context(tc.tile_pool(name="consts", bufs=1))
        wpool = ctx.enter_context(tc.tile_pool(name="wpool", bufs=1))
        wmov = ctx.enter_context(tc.tile_pool(name="wmov", bufs=6))
        xp = ctx.enter_context(tc.tile_pool(name="xp", bufs=2))
        qk = ctx.enter_context(tc.tile_pool(name="qk", bufs=1))
        va_p = ctx.enter_context(tc.tile_pool(name="va", bufs=2))
        ot_p = ctx.enter_context(tc.tile_pool(name="ot", bufs=2))
        pt_p = ctx.enter_context(tc.tile_pool(name="pt", bufs=2))
        rec_p = ctx.enter_context(tc.tile_pool(name="rec", bufs=2))
        stg_p = ctx.enter_context(tc.tile_pool(name="stg", bufs=2))
        osb_p = ctx.enter_context(tc.tile_pool(name="osb", bufs=2))
        scr_p = ctx.enter_context(tc.tile_pool(name="scr", bufs=4, space="DRAM"))
        ps = ctx.enter_context(tc.tile_pool(name="ps", bufs=2, space="PSUM"))

        # ---- constants & resident weights (once) ------------------------
        bq_sb = consts.tile([128, NT], F32, tag="bq")
        mask_sb = consts.tile([128, 128], BF16, tag="mask")
        bo_bc = consts.tile([128, D], F32, tag="bobc")
        ones_sb = consts.tile([128, DK], BF16, tag="ones")
        nc.vector.memset(ones_sb, 1.0)
        nc.sync.dma_start(out=bq_sb, in_=bq_d[:, :])
        nc.sync.dma_start(out=mask_sb, in_=mask_d[:, :])
        bop_bcast = bass.AP(tensor=bop_d, offset=0, ap=[[0, 128], [1, D]])
        nc.sync.dma_start(out=bo_bc, in_=bop_bcast)


        wce_pre = {}
        for w_d, nm in ((wq_d, "q"), (wk_d, "k")):
            wce = wmov.tile([128, NT, 128], BF16, name="wqkc", tag="wqkc",
                            bufs=3)
            nc.sync.dma_start(out=wce, in_=w_d[0])
            wce_pre[nm] = wce

        xts = []
        for blk in range(NB):
            xt = xp.tile([128, NT, L], BF16, name=f"xt{blk}", tag="xt")
            nc.sync.dma_start(out=xt[:, 0:4, :], in_=xt_d[blk, :, 0:4, :])
            nc.sync.dma_start(out=xt[:, 4:8, :], in_=xt_d[blk, :, 4:8, :])
            xts.append(xt)

        fill_units = []

        for blk in range(NB):
            xt = xts[blk]

            # ---- Q/K projections, out [e, l] bf16, per-et tiles ---------
            qts, kts = [], []
            for et in range(NT):
                for w_d, b_ap, lst, nm in ((wq_d, bq_sb, qts, "q"),
                                           (wk_d, None, kts, "k")):
                    o_sb = qk.tile([128, L], BF16, name=f"{nm}t{et}",
                                   tag=f"{nm}t{et}")
                    lst.append(o_sb)
                    if blk == 0 and et == 0:
                        wce = wce_pre[nm]
                    else:
                        wce = wmov.tile([128, NT, 128], BF16, name="wqkc",
                                        tag="wqkc", bufs=3)
                        nc.sync.dma_start(out=wce, in_=w_d[et])
                    for j in range(2):
                        psq = ps.tile([128, 512], F32, tag="psa", bufs=2)
                        for d in range(NT):
                            nc.tensor.matmul(
                                psq[:, :], wce[:, d, :],
                                xt[:, d, j * 512:(j + 1) * 512],
                                start=(d == 0), stop=(d == NT - 1))
                        dst = o_sb[:, j * 512:(j + 1) * 512]
                        if b_ap is not None:
                            nc.vector.tensor_scalar_add(
                                out=dst, in0=psq, scalar1=b_ap[:, et:et + 1])
                        else:
                            nc.vector.tensor_copy(out=dst, in_=psq)

            # ---- V projection into V_aug bf16 ---------------------------
            # va[:, lt, h, 0:64] = 32*v[lk, e in head h]; col 64 = 32.0
            va = va_p.tile([128, NLT, H, DK + 1], BF16, tag="va")
            nc.vector.memset(va[:, :, :, DK:DK + 1], WS)
            for g in range(2):
                chunks = []
                for d in range(NT):
                    wc = wmov.tile([128, 512], BF16, tag="wvc", bufs=10)
                    nc.sync.dma_start(out=wc, in_=wv_d[:, d, g * 512:(g + 1) * 512])
                    chunks.append(wc)
                for lt in range(NLT):
                    psv = ps.tile([128, 512], F32, tag="psa", bufs=2)
                    for d in range(NT):
                        nc.tensor.matmul(
                            psv[:, :], xt[:, d, lt * 128:(lt + 1) * 128],
                            chunks[d][:, :],
                            start=(d == 0), stop=(d == NT - 1))
                    nc.scalar.copy(
                        out=va[:, lt, g * 8:(g + 1) * 8, 0:DK],
                        in_=psv.rearrange("p (h k) -> p h k", h=8))

            # ---- attention ---------------------------------------------
            ot = ot_p.tile([128, NT, L], BF16, tag="ot")
            pending_nrm = []

            def nrm_stage():
                # PE broadcast of the reciprocal'd denominator row (K=1
                # f32r matmul: ones x row), then normalize out of psum.
                # Runs one unit behind its PV so the PE never waits on
                # the DVE reciprocal.
                h, j, op, rrow, rec = pending_nrm.pop(0)
                et, half = h // 2, h % 2
                p0 = DK * half
                rbc = ps.tile([DK, 512], F32, tag="rbc", bufs=1)
                nc.tensor.matmul(
                    rbc[:, :],
                    ones_sb[DK:DK + 1, 0:DK],
                    rrow[DK:DK + 1, :],
                    start=True, stop=True)
                nc.vector.tensor_copy(out=rec[0:DK, :], in_=rbc[:, :])
                dst_cols = slice(j * 512, (j + 1) * 512)
                if half == 0:
                    nc.vector.tensor_mul(
                        out=ot[0:DK, et, dst_cols],
                        in0=op[0:DK, :], in1=rec[0:DK, :])
                else:
                    stg = stg_p.tile([DK, 512], BF16, tag="stg")
                    nc.vector.tensor_mul(out=stg, in0=op[0:DK, :],
                                         in1=rec[0:DK, :])
                    nc.sync.dma_start(
                        out=ot[p0:p0 + DK, et, dst_cols], in_=stg)

            def pv_stage(h, j, pt):
                if pending_nrm:
                    nrm_stage()
                op = ps.tile([DK + 1, 512], F32, tag="pvs", bufs=2)
                tiles = [(i, max(0, 128 * (i - 4 * j))) for i in range(4 * j + 4)]
                for n, (i, lo) in enumerate(tiles):
                    nc.tensor.matmul(
                        op[0:DK + 1, lo:512],
                        va[:, i, h, 0:DK + 1],
                        pt[:, i, lo:512],
                        start=(n == 0), stop=(n == len(tiles) - 1))
                rrow = rec_p.tile([DK + 1, 512], BF16, tag="rrow")
                rec = rec_p.tile([DK, 512], F32, tag="rec")
                with nc.allow_low_precision("bf16 1/denominator (<=0.4%)"):
                    nc.vector.reciprocal(out=rrow[DK:DK + 1, :],
                                         in_=op[DK:DK + 1, :])
                pending_nrm.append((h, j, op, rrow, rec))

            pending = []
            for h in range(H):
                et, half = h // 2, h % 2
                p0 = DK * half
                qt, kt = qts[et], kts[et]
                pts = (pt_p.tile([128, 4, 512], BF16, name="pt0", tag="pt0", bufs=3),
                       pt_p.tile([128, NLT, 512], BF16, name="pt1", tag="pt1",
                                 bufs=3))
                for j in range(2):
                    pt = pts[j]
                    # diagonal band first so the mask can fire early
                    order = list(range(4 * j, 4 * j + 4)) + list(range(4 * j))
                    for n_i, i in enumerate(order):
                        d = i - 4 * j
                        lo = 128 * d if d > 0 else 0
                        sp = ps.tile([128, 512], F32, tag="sps", bufs=3)
                        nc.tensor.matmul(
                            sp[:, lo:512],
                            kt[p0:p0 + DK, i * 128:(i + 1) * 128],
                            qt[p0:p0 + DK, j * 512 + lo:(j + 1) * 512],
                            start=True, stop=True)
                        nc.scalar.activation(
                            out=pt[:, i, lo:512], in_=sp[:, lo:512],
                            func=mybir.ActivationFunctionType.Exp,
                            scale=EXPS)
                        if n_i == 3:
                            # mask the 4 diagonal tiles (offset of diag
                            # block i: 512*i + 128*(i-4j), stride 640)
                            diag = bass.AP(
                                tensor=pt.tensor,
                                offset=pt.offset + (2048 * j),
                                ap=[pt.ap[0], [640, 4], [1, 128]])
                            mask_bc = bass.AP(
                                tensor=mask_sb.tensor,
                                offset=mask_sb.offset,
                                ap=[mask_sb.ap[0], [0, 4], [1, 128]])
                            nc.vector.tensor_mul(out=diag, in0=diag,
                                                 in1=mask_bc)
                    # PV of the previous unit runs behind this unit's
                    # scores so the PE never waits on exp/mask
                    if pending:
                        pv_stage(*pending.pop(0))
                    pending.append((h, j, pt))
                if fill_units and h >= 4:
                    fill_units.pop(0)()
            while pending:
                pv_stage(*pending.pop(0))
            while pending_nrm:
                nrm_stage()
            while fill_units:
                fill_units.pop(0)()

            # ---- output projection (bf16) ------------------------------
            # g=0 runs now; g=1 of a non-final block is deferred into the
            # next block's attention phase to fill PE bubbles there.
            def op_chunks(g):
                chunks = []
                for et in range(NT):
                    wc = wmov.tile([128, 512], BF16, tag="woc", bufs=10)
                    nc.sync.dma_start(out=wc,
                                      in_=wo_d[:, et, g * 512:(g + 1) * 512])
                    chunks.append(wc)
                return chunks

            def op_unit(blk_, ot_, chunks, g, lt):
                po = ps.tile([128, 512], F32, tag="psa", bufs=2)
                for et in range(NT):
                    nc.tensor.matmul(
                        po[:, :], ot_[:, et, lt * 128:(lt + 1) * 128],
                        chunks[et][:, :],
                        start=(et == 0), stop=(et == NT - 1))
                osb = osb_p.tile([128, 512], F32, tag="osb")
                nc.vector.tensor_add(out=osb, in0=po,
                                     in1=bo_bc[:, g * 512:(g + 1) * 512])
                nc.sync.dma_start(
                    out=out_d[blk_, lt, :, g * 512:(g + 1) * 512],
                    in_=osb)

            chunks0 = op_chunks(0)
            for lt in range(NLT):
                op_unit(blk, ot, chunks0, 0, lt)
            if blk < NB - 1:
                state = {}

                def fill0(blk_=blk, ot_=ot, state=state):
                    state["c"] = op_chunks(1)
                    op_unit(blk_, ot_, state["c"], 1, 0)

                fill_units.append(fill0)
                for lt in range(1, NLT):
                    def fill(blk_=blk, ot_=ot, state=state, lt_=lt):
                        op_unit(blk_, ot_, state["c"], 1, lt_)
                    fill_units.append(fill)
            else:
                chunks1 = op_chunks(1)
                for lt in range(NLT):
                    op_unit(blk, ot, chunks1, 1, lt)
    nc.compile()
    return nc


def _prep_core_inputs(core, x, Wq, Wk, Wv, Wo, bq, bk, bv, bo, mask_bf):
    bf = ml_dtypes.bfloat16
    s = core // 2
    bs = (2 * (core % 2), 2 * (core % 2) + 1)
    xt = np.empty((NB, 128, NT, L), bf)
    for n, b in enumerate(bs):
        xb = np.ascontiguousarray(x[b, s * L:(s + 1) * L, :].T)      # [D, L]
        xt[n] = xb.reshape(NT, 128, L).transpose(1, 0, 2).astype(bf)
    # stationary W tiles: wq[p, d, et, m] = 32*Wq[d*128+p, et*128+m]
    wq = np.ascontiguousarray(
        (Wq[s] * WS).reshape(NT, 128, NT, 128).transpose(2, 1, 0, 3)).astype(bf)
    wk = np.ascontiguousarray(
        (Wk[s] * WS).reshape(NT, 128, NT, 128).transpose(2, 1, 0, 3)).astype(bf)
    wv = np.ascontiguousarray(
        (Wv[s] * WS).reshape(NT, 128, D).transpose(1, 0, 2)).astype(bf)
    wo = np.ascontiguousarray(
        Wo[s].reshape(NT, 128, D).transpose(1, 0, 2)).astype(bf)
    bq_t = np.ascontiguousarray((bq[s] * WS).reshape(NT, 128).T).astype(np.float32)
    bop = (bv[s] @ Wo[s] + bo[s]).reshape(1, D).astype(np.float32)
    return {"xt": xt, "wq": wq, "wk": wk, "wv": wv, "wo": wo,
            "bq": bq_t, "bop": bop, "mask": mask_bf}


_PROGRAM_CACHE = {}


def run(x, Wq, Wk, Wv, Wo, bq, bk, bv, bo, trace=False, **run_kwargs):
    x = np.asarray(x, np.float32)
    Wq, Wk, Wv, Wo = (np.asarray(a, np.float32) for a in (Wq, Wk, Wv, Wo))
    bq, bk, bv, bo = (np.asarray(a, np.float32) for a in (bq, bk, bv, bo))
    mask_bf = np.triu(np.ones((128, 128))).astype(ml_dtypes.bfloat16)

    if "nc" not in _PROGRAM_CACHE:
        _PROGRAM_CACHE["nc"] = build_program()
    nc = _PROGRAM_CACHE["nc"]

    in_maps = [_prep_core_inputs(c, x, Wq, Wk, Wv, Wo, bq, bk, bv, bo, mask_bf)
               for c in range(NCORES)]
    res = run_bass_kernel_spmd(nc, in_maps, core_ids=list(range(NCORES)),
                               trace=trace, **run_kwargs)
    out = np.empty((B, SEQ, D), np.float32)
    for c in range(NCORES):
        s = c // 2
        for n, b in enumerate((2 * (c % 2), 2 * (c % 2) + 1)):
            out[b, s * L:(s + 1) * L, :] = res.results[c]["out"][n].reshape(L, D)
    return out, res


def kernel(x, Wq, Wk, Wv, Wo, bq, bk, bv, bo):
    out, _ = run(x, Wq, Wk, Wv, Wo, bq, bk, bv, bo, trace=False)
    return out


# revision 41
# speedup vs baseline: 1.3659x; 1.0092x over previous
"""Bass/Trainium2 kernel for LocalAttention (block-diagonal MHA, causal).

Model: x[B=4, SEQ=4096, D=1024] split into SPLIT=4 sequence blocks of L=1024,
each with its own MHA weights (H=16 heads, DK=64), causal within block.

Sharding: 16 (batch, split) blocks over 8 cores; core i handles split i//2 and
batches {2*(i%2), 2*(i%2)+1}. No collectives.

Per-core program (v3, precision-safe bf16 compute):
  - wq/wk bulk-loaded once; wv/wo streamed in 512-col chunks per block.
  - Q/K projections: W stationary, x^T moving, out qt/kt [e, l] bf16 per-et
    tiles (Q bias fused into the PSUM->SBUF copy; K bias dropped - it is a
    per-query constant, softmax-invariant).
  - Scores q^T k bf16 per lk-tile, exact causal trim; exp on ACT with fused
    scale 1/(32*32*sqrt(DK)) writing bf16 P^T; diagonal blocks masked by one
    strided bf16 multiply per (head, j).
  - PV bf16 with a 32.0 ones-column in V_aug giving the softmax denominator
    in psum row 64; denominator row -> reciprocal -> gpsimd partition
    broadcast; the normalize-multiply doubles as the PSUM->SBUF bf16 move
    and is software-pipelined one unit behind (never holds the DVE queue).
  - PV of head h runs behind the scores of head h+1 so the PE stream never
    waits on exp/mask.
  - Out-projection bf16; bias bo' = bv@Wo + bo added via broadcast row.
"""

import numpy as np
import ml_dtypes
from contextlib import ExitStack

import concourse.bass as bass
import concourse.bacc as bacc
import concourse.mybir as mybir
import concourse.tile as tile
from concourse.bass_utils import run_bass_kernel_spmd

H = 16
D = 1024
SPLIT = 4
DK = 64
B = 4
SEQ = 4096
L = SEQ // SPLIT          # 1024 tokens per block
NB = 2                    # blocks per core
NCORES = 8
NT = D // 128             # 8 e-tiles / d-tiles
NLT = L // 128            # 8 lk-tiles
F32 = mybir.dt.float32
BF16 = mybir.dt.bfloat16
WS = 32.0                 # q/k scale (folded out in the exp scale)
EXPS = 1.0 / (WS * WS * 8.0)


def build_program():
    nc = bacc.Bacc()

    xt_d = nc.declare_dram_parameter("xt", [NB, 128, NT, L], BF16, isOutput=False)
    wq_d = nc.declare_dram_parameter("wq", [NT, 128, NT, 128], BF16, isOutput=False)
    wk_d = nc.declare_dram_parameter("wk", [NT, 128, NT, 128], BF16, isOutput=False)
    wv_d = nc.declare_dram_parameter("wv", [128, NT, D], BF16, isOutput=False)
    wo_d = nc.declare_dram_parameter("wo", [128, NT, D], BF16, isOutput=False)
    bq_d = nc.declare_dram_parameter("bq", [128, NT], F32, isOutput=False)
    bop_d = nc.declare_dram_parameter("bop", [1, D], F32, isOutput=False)
    mask_d = nc.declare_dram_parameter("mask", [128, 128], BF16, isOutput=False)
    out_d = nc.declare_dram_parameter("out", [NB, NLT, 128, D], F32, isOutput=True)

    with ExitStack() as ctx:
        tc = ctx.enter_context(tile.TileContext(nc))
        consts = ctx.enter_context(tc.tile_pool(name="consts", bufs=1))
        wpool = ctx.enter_context(tc.tile_pool(name="wpool", bufs=1))
        wmov = ctx.enter_context(tc.tile_pool(name="wmov", bufs=6))
        xp = ctx.enter_context(tc.tile_pool(name="xp", bufs=2))
        qk = ctx.enter_context(tc.tile_pool(name="qk", bufs=1))
        va_p = ctx.enter_context(tc.tile_pool(name="va", bufs=2))
        ot_p = ctx.enter_context(tc.tile_pool(name="ot", bufs=2))
        pt_p = ctx.enter_context(tc.tile_pool(name="pt", bufs=2))
        rec_p = ctx.enter_context(tc.tile_pool(name="rec", bufs=2))
        stg_p = ctx.enter_context(tc.tile_pool(name="stg", bufs=2))
        osb_p = ctx.enter_context(tc.tile_pool(name="osb", bufs=2))
        scr_p = ctx.enter_context(tc.tile_pool(name="scr", bufs=4, space="DRAM"))
        ps = ctx.enter_context(tc.tile_pool(name="ps", bufs=2, space="PSUM"))

        # ---- constants & resident weights (once) ------------------------
        bq_sb = consts.tile([128, NT], F32, tag="bq")
        mask_sb = consts.tile([128, 128], BF16, tag="mask")
        bo_bc = consts.tile([128, D], F32, tag="bobc")
        ones_sb = consts.tile([128, DK], BF16, tag="ones")
        nc.vector.memset(ones_sb, 1.0)
        nc.sync.dma_start(out=bq_sb, in_=bq_d[:, :])
        nc.sync.dma_start(out=mask_sb, in_=mask_d[:, :])
        bop_bcast = bass.AP(tensor=bop_d, offset=0, ap=[[0, 128], [1, D]])
        nc.sync.dma_start(out=bo_bc, in_=bop_bcast)


        wce_pre = {}
        for w_d, nm in ((wq_d, "q"), (wk_d, "k")):
            wce = wmov.tile([128, NT, 128], BF16, name="wqkc", tag="wqkc",
                            bufs=3)
            nc.sync.dma_start(out=wce, in_=w_d[0])
            wce_pre[nm] = wce

        xts = []
        for blk in range(NB):
            xt = xp.tile([128, NT, L], BF16, name=f"xt{blk}", tag="xt")
            nc.sync.dma_start(out=xt[:, 0:4, :], in_=xt_d[blk, :, 0:4, :])
            nc.sync.dma_start(out=xt[:, 4:8, :], in_=xt_d[blk, :, 4:8, :])
            xts.append(xt)

        fill_units = []

        for blk in range(NB):
            xt = xts[blk]

            # ---- Q/K projections, out [e, l] bf16, per-et tiles ---------
            qts, kts = [], []
            for et in range(NT):
                for w_d, b_ap, lst, nm in ((wq_d, bq_sb, qts, "q"),
                                           (wk_d, None, kts, "k")):
                    o_sb = qk.tile([128, L], BF16, name=f"{nm}t{et}",
                                   tag=f"{nm}t{et}")
                    lst.append(o_sb)
                    if blk == 0 and et == 0:
                        wce = wce_pre[nm]
                    else:
                        wce = wmov.tile([128, NT, 128], BF16, name="wqkc",
                                        tag="wqkc", bufs=3)
                        nc.sync.dma_start(out=wce, in_=w_d[et])
                    for j in range(2):
                        psq = ps.tile([128, 512], F32, tag="psa", bufs=2)
                        for d in range(NT):
                            nc.tensor.matmul(
                                psq[:, :], wce[:, d, :],
                                xt[:, d, j * 512:(j + 1) * 512],
                                start=(d == 0), stop=(d == NT - 1))
                        dst = o_sb[:, j * 512:(j + 1) * 512]
                        if b_ap is not None:
                            nc.vector.tensor_scalar_add(
                                out=dst, in0=psq, scalar1=b_ap[:, et:et + 1])
                        else:
                            nc.vector.tensor_copy(out=dst, in_=psq)

            # ---- V projection into V_aug bf16 ---------------------------
            # va[:, lt, h, 0:64] = 32*v[lk, e in head h]; col 64 = 32.0
            va = va_p.tile([128, NLT, H, DK + 1], BF16, tag="va")
            nc.vector.memset(va[:, :, :, DK:DK + 1], WS)
            for g in range(2):
                chunks = []
                for d in range(NT):
                    wc = wmov.tile([128, 512], BF16, tag="wvc", bufs=10)
                    nc.sync.dma_start(out=wc, in_=wv_d[:, d, g * 512:(g + 1) * 512])
                    chunks.append(wc)
                for lt in range(NLT):
                    psv = ps.tile([128, 512], F32, tag="psa", bufs=2)
                    for d in range(NT):
                        nc.tensor.matmul(
                            psv[:, :], xt[:, d, lt * 128:(lt + 1) * 128],
                            chunks[d][:, :],
                            start=(d == 0), stop=(d == NT - 1))
                    nc.scalar.copy(
                        out=va[:, lt, g * 8:(g + 1) * 8, 0:DK],
                        in_=psv.rearrange("p (h k) -> p h k", h=8))

            # ---- attention ---------------------------------------------
            ot = ot_p.tile([128, NT, L], BF16, tag="ot")
            pending_nrm = []

            def nrm_stage():
                # PE broadcast of the reciprocal'd denominator row (K=1
                # f32r matmul: ones x row), then normalize out of psum.
                # Runs one unit behind its PV so the PE never waits on
                # the DVE reciprocal.
                h, j, op, rrow, rec = pending_nrm.pop(0)
                et, half = h // 2, h % 2
                p0 = DK * half
                rbc = ps.tile([DK, 512], F32, tag="rbc", bufs=1)
                nc.tensor.matmul(
                    rbc[:, :],
                    ones_sb[DK:DK + 1, 0:DK],
                    rrow[DK:DK + 1, :],
                    start=True, stop=True)
                nc.vector.tensor_copy(out=rec[0:DK, :], in_=rbc[:, :])
                dst_cols = slice(j * 512, (j + 1) * 512)
                if half == 0:
                    nc.vector.tensor_mul(
                        out=ot[0:DK, et, dst_cols],
                        in0=op[0:DK, :], in1=rec[0:DK, :])
                else:
                    stg = stg_p.tile([DK, 512], BF16, tag="stg")
                    nc.vector.tensor_mul(out=stg, in0=op[0:DK, :],
                                         in1=rec[0:DK, :])
                    nc.sync.dma_start(
                        out=ot[p0:p0 + DK, et, dst_cols], in_=stg)

            def pv_stage(h, j, pt):
                if pending_nrm:
                    nrm_stage()
                op = ps.tile([DK + 1, 512], F32, tag="pvs", bufs=2)
                tiles = [(i, max(0, 128 * (i - 4 * j))) for i in range(4 * j + 4)]
                for n, (i, lo) in enumerate(tiles):
                    nc.tensor.matmul(
                        op[0:DK + 1, lo:512],
                        va[:, i, h, 0:DK + 1],
                        pt[:, i, lo:512],
                        start=(n == 0), stop=(n == len(tiles) - 1))
                rrow = rec_p.tile([DK + 1, 512], BF16, tag="rrow")
                rec = rec_p.tile([DK, 512], F32, tag="rec")
                with nc.allow_low_precision("bf16 1/denominator (<=0.4%)"):
                    nc.vector.reciprocal(out=rrow[DK:DK + 1, :],
                                         in_=op[DK:DK + 1, :])
                pending_nrm.append((h, j, op, rrow, rec))

            pending = []
            head_order = [h ^ 1 for h in range(H)]
            for h in head_order:
                et, half = h // 2, h % 2
                p0 = DK * half
                qt, kt = qts[et], kts[et]
                pts = (pt_p.tile([128, 4, 512], BF16, name="pt0", tag="pt0", bufs=3),
                       pt_p.tile([128, NLT, 512], BF16, name="pt1", tag="pt1",
                                 bufs=3))
                for j in range(2):
                    pt = pts[j]
                    # diagonal band first so the mask can fire early
                    order = list(range(4 * j, 4 * j + 4)) + list(range(4 * j))
                    for n_i, i in enumerate(order):
                        d = i - 4 * j
                        lo = 128 * d if d > 0 else 0
                        sp = ps.tile([128, 512], F32, tag="sps", bufs=3)
                        nc.tensor.matmul(
                            sp[:, lo:512],
                            kt[p0:p0 + DK, i * 128:(i + 1) * 128],
                            qt[p0:p0 + DK, j * 512 + lo:(j + 1) * 512],
                            start=True, stop=True)
                        nc.scalar.activation(
                            out=pt[:, i, lo:512], in_=sp[:, lo:512],
                            func=mybir.ActivationFunctionType.Exp,
                            scale=EXPS)
                        if n_i == 3:
                            # mask the 4 diagonal tiles (offset of diag
                            # block i: 512*i + 128*(i-4j), stride 640)
                            diag = bass.AP(
                                tensor=pt.tensor,
                                offset=pt.offset + (2048 * j),
                                ap=[pt.ap[0], [640, 4], [1, 128]])
                            mask_bc = bass.AP(
                                tensor=mask_sb.tensor,
                                offset=mask_sb.offset,
                                ap=[mask_sb.ap[0], [0, 4], [1, 128]])
                            nc.vector.tensor_mul(out=diag, in0=diag,
                                                 in1=mask_bc)
                    # PV of the previous unit runs behind this unit's
                    # scores so the PE never waits on exp/mask
                    if pending:
                        pv_stage(*pending.pop(0))
                    pending.append((h, j, pt))
                if fill_units and h >= 4:
                    fill_units.pop(0)()
            while pending:
                pv_stage(*pending.pop(0))
            while pending_nrm:
                nrm_stage()
            while fill_units:
                fill_units.pop(0)()

            # ---- output projection (bf16) ------------------------------
            # g=0 runs now; g=1 of a non-final block is deferred into the
            # next block's attention phase to fill PE bubbles there.
            def op_chunks(g):
                chunks = []
                for et in range(NT):
                    wc = wmov.tile([128, 512], BF16, tag="woc", bufs=10)
                    nc.sync.dma_start(out=wc,
                                      in_=wo_d[:, et, g * 512:(g + 1) * 512])
                    chunks.append(wc)
                return chunks

            def op_unit(blk_, ot_, chunks, g, lt):
                po = ps.tile([128, 512], F32, tag="psa", bufs=2)
                for et in range(NT):
                    nc.tensor.matmul(
                        po[:, :], ot_[:, et, lt * 128:(lt + 1) * 128],
                        chunks[et][:, :],
                        start=(et == 0), stop=(et == NT - 1))
                osb = osb_p.tile([128, 512], F32, tag="osb")
                nc.vector.tensor_add(out=osb, in0=po,
                                     in1=bo_bc[:, g * 512:(g + 1) * 512])
                nc.sync.dma_start(
                    out=out_d[blk_, lt, :, g * 512:(g + 1) * 512],
                    in_=osb)

            chunks0 = op_chunks(0)
            for lt in range(NLT):
                op_unit(blk, ot, chunks0, 0, lt)
            if blk < NB - 1:
                state = {}

                def fill0(blk_=blk, ot_=ot, state=state):
                    state["c"] = op_chunks(1)
                    op_unit(blk_, ot_, state["c"], 1, 0)

                fill_units.append(fill0)
                for lt in range(1, NLT):
                    def fill(blk_=blk, ot_=ot, state=state, lt_=lt):
                        op_unit(blk_, ot_, state["c"], 1, lt_)
                    fill_units.append(fill)
            else:
                chunks1 = op_chunks(1)
                for lt in range(NLT):
                    op_unit(blk, ot, chunks1, 1, lt)
    nc.compile()
    return nc


def _prep_core_inputs(core, x, Wq, Wk, Wv, Wo, bq, bk, bv, bo, mask_bf):
    bf = ml_dtypes.bfloat16
    s = core // 2
    bs = (2 * (core % 2), 2 * (core % 2) + 1)
    xt = np.empty((NB, 128, NT, L), bf)
    for n, b in enumerate(bs):
        xb = np.ascontiguousarray(x[b, s * L:(s + 1) * L, :].T)      # [D, L]
        xt[n] = xb.reshape(NT, 128, L).transpose(1, 0, 2).astype(bf)
    # stationary W tiles: wq[p, d, et, m] = 32*Wq[d*128+p, et*128+m]
    wq = np.ascontiguousarray(
        (Wq[s] * WS).reshape(NT, 128, NT, 128).transpose(2, 1, 0, 3)).astype(bf)
    wk = np.ascontiguousarray(
        (Wk[s] * WS).reshape(NT, 128, NT, 128).transpose(2, 1, 0, 3)).astype(bf)
    wv = np.ascontiguousarray(
        (Wv[s] * WS).reshape(NT, 128, D).transpose(1, 0, 2)).astype(bf)
    wo = np.ascontiguousarray(
        Wo[s].reshape(NT, 128, D).transpose(1, 0, 2)).astype(bf)
    bq_t = np.ascontiguousarray((bq[s] * WS).reshape(NT, 128).T).astype(np.float32)
    bop = (bv[s] @ Wo[s] + bo[s]).reshape(1, D).astype(np.float32)
    return {"xt": xt, "wq": wq, "wk": wk, "wv": wv, "wo": wo,
            "bq": bq_t, "bop": bop, "mask": mask_bf}


_PROGRAM_CACHE = {}


def run(x, Wq, Wk, Wv, Wo, bq, bk, bv, bo, trace=False, **run_kwargs):
    x = np.asarray(x, np.float32)
    Wq, Wk, Wv, Wo = (np.asarray(a, np.float32) for a in (Wq, Wk, Wv, Wo))
    bq, bk, bv, bo = (np.asarray(a, np.float32) for a in (bq, bk, bv, bo))
    mask_bf = np.triu(np.ones((128, 128))).astype(ml_dtypes.bfloat16)

    if "nc" not in _PROGRAM_CACHE:
        _PROGRAM_CACHE["nc"] = build_program()
    nc = _PROGRAM_CACHE["nc"]

    in_maps = [_prep_core_inputs(c, x, Wq, Wk, Wv, Wo, bq, bk, bv, bo, mask_bf)
               for c in range(NCORES)]
    res = run_bass_kernel_spmd(nc, in_maps, core_ids=list(range(NCORES)),
                               trace=trace, **run_kwargs)
    out = np.empty((B, SEQ, D), np.float32)
    for c in range(NCORES):
        s = c // 2
        for n, b in enumerate((2 * (c % 2), 2 * (c % 2) + 1)):
            out[b, s * L:(s + 1) * L, :] = res.results[c]["out"][n].reshape(L, D)
    return out, res


def kernel(x, Wq, Wk, Wv, Wo, bq, bk, bv, bo):
    out, _ = run(x, Wq, Wk, Wv, Wo, bq, bk, bv, bo, trace=False)
    return out
